# revision 16
# baseline (speedup 1.0000x reference)
"""Trainium2 Bass kernel for nn_CIFAR10_Monarch_MLP2 (4-layer Monarch MLP + log_softmax).

Strategy
--------
Data-parallel over 8 NeuronCores: each core computes 2048 rows of the
16384-row batch with replicated weights; outputs are concatenated on host.

Per core, activations are kept feature-major ([feature partitions, batch
free]).  x is pre-cast to bf16 and pre-transposed on the host, so the device
just DMAs feature-major batch tiles (NB=512 columns) straight into SBUF —
no on-device cast or transpose.

The monarch permutation (flat index k*q+qq -> plane l=(f%4), row r=f//4) is
folded into a host-side re-arrangement of the weights:

 * w1 rows of block k are regrouped by destination plane l, each group padded
   to a fixed `chunk` (multiple of 32).  mm1 then runs natural M=128 tiles
   and its PSUM evictions scatter fragments into the plane layout with DVE
   copies, using the hardware's partition-shift capability (64-sized copies
   between halves, 32-sized between quadrants).
 * w2 columns are permuted to match the resulting plane-row order (pad rows
   get zero columns), so no data movement is needed for the permutation.
 * Layers 3/4 are tiny: both are collapsed on the host into small dense
   matrices (block-diag x permutation x block-diag composed exactly), so L3
   is 8 accumulating matmuls into one [100, NB] PSUM tile and L4 is an
   operand-swapped [batch, 10] matmul + bias whose output lands batch-major
   for a cheap free-dim log_softmax.

Weights are host-preswizzled into flat images matching the SBUF tile layout
exactly, so the device loads them with ~11 large (0.2-4.6 MB) DMAs instead
of ~96 small descriptor-dominated ones: L1/L2 w1 stream per-block on the
ACT HWDGE ring (consumed in that order by the m-major mm1), while biases
(needed by the first relu eviction), the 4.6 MB L1 w2 image (needed when
mm2 starts), L2 w2 and the packed w3/L4 constants go on the gpsimd SWDGE
ring — two rings fill in parallel under tile-0/1 compute, and the ACT
engine queue stays clear of descriptor generation for the evictions.
x tiles stream on the SP queue, double-buffered against compute.  mm1 runs
m-major (k inner) so mm2's plane dependencies complete progressively.
Relu+bias evictions are split between ACT (activation) and DVE (fused
tensor_scalar add+max) to balance engine load; all activation functions are
pinned to one act-table set so no per-tile table swaps occur.  Each tile's
L4+softmax tail is emitted inside the NEXT tile's first matmul group so the
serial tail rides where ACT/DVE are idle; log_softmax skips the max-shift
(logits are bounded, fp32 exp cannot overflow), shortening the exposed
final-tile chain.
"""

import functools

import numpy as np
import ml_dtypes

import concourse.bass as bass
from concourse import bacc
import concourse.mybir as mybir
import concourse.tile as tile
from concourse.bass_utils import run_bass_kernel_spmd

# Pin all our activation functions (Relu/Exp/Ln/Identity/Copy) to the one
# act-table set that contains them all ("natural_log_exp_and_others"), so the
# act-table-load fixpoint emits a single load instead of swapping tables
# (1.3us each, on the ACT critical path) between relu evictions and the
# softmax tail every batch tile.  Canonical set order/names are preserved, so
# the emitted act_func_set_id still indexes the real act_info.json.
_PIN_SET = "natural_log_exp_and_others"
_orig_get_tables = bacc.get_activation_tables


@functools.cache
def _pinned_tables(arch):
    tabs = _orig_get_tables(arch)
    AF = mybir.ActivationFunctionType
    ours = {AF.Relu, AF.Exp, AF.Ln, AF.Identity, AF.Copy}
    if _PIN_SET not in tabs or not ours <= tabs[_PIN_SET]:
        return tabs
    return {
        name: (funcs if name == _PIN_SET else funcs - ours)
        for name, funcs in tabs.items()
    }


bacc.get_activation_tables = _pinned_tables

F32 = mybir.dt.float32
MM_DT = mybir.dt.bfloat16

N_CORES = 8
BATCH = 16384
B_CORE = BATCH // N_CORES  # 2048
NB = 512  # batch-tile free size (PSUM bank = 2KB -> 512 fp32)

# (p_dim, q_dim, chunk, s_dim) for the two big monarch layers
LAYER_CFG = [
    (768, 750, 192, 750),
    (750, 250, 64, 250),
]

# per-group engine for relu+bias evictions: 'A' = ACT activation,
# 'V' = DVE fused tensor_scalar, 'P' = gpsimd fused tensor_scalar
L1_RELU_ENG = ("A" * 5 + "V") * 4       # 24 groups: 20 ACT, 4 DVE
L2_RELU_ENG = "AAVAAVAA"                # 8 groups: 6 ACT, 2 DVE
L3_RELU_ENG = "A"


def _np_mmdt():
    return {
        mybir.dt.bfloat16: ml_dtypes.bfloat16,
        mybir.dt.float32r: np.float32,
        mybir.dt.float32: np.float32,
    }[MM_DT]


def arrange_layer(w1, w2, q_dim, chunk):
    """w1:(4,q,p), w2:(4,s,r=q) -> w1t:[4,p,QPAD] (mm1 lhsT), w2t:[4,QPAD,s]
    (mm2 lhsT), with the monarch permutation folded in (see module doc)."""
    nb, _, p_dim = w1.shape
    s_dim = w2.shape[1]
    QPAD = 4 * chunk
    w1t = np.zeros((nb, p_dim, QPAD), np.float32)
    w2t = np.zeros((nb, QPAD, s_dim), np.float32)
    for k in range(nb):
        for l in range(nb):
            qs = [q for q in range(q_dim) if (k * q_dim + q) % 4 == l]
            w1t[k, :, l * chunk : l * chunk + len(qs)] = w1[k, qs, :].T
            rs = [(k * q_dim + q) // 4 for q in qs]
            w2t[l, k * chunk : k * chunk + len(qs), :] = w2[l, :, rs]
    return w1t, w2t


def compose_monarch(w1, w2, out_features):
    """Exact dense matrix of a MonarchLinear (no bias): [out_features, fin]."""
    k, q, p = w1.shape
    l, s, r = w2.shape
    W1big = np.zeros((k * q, k * p), np.float64)
    for kk in range(k):
        W1big[kk * q : (kk + 1) * q, kk * p : (kk + 1) * p] = w1[kk]
    P = np.zeros((l * r, k * q), np.float64)
    for ll in range(l):
        for rr in range(r):
            P[ll * r + rr, rr * l + ll] = 1.0
    W2big = np.zeros((l * s, l * r), np.float64)
    for ll in range(l):
        W2big[ll * s : (ll + 1) * s, ll * r : (ll + 1) * r] = w2[ll]
    M = W2big @ P @ W1big
    return M[:out_features].astype(np.float32)


def evict_frags(k, m, chunk):
    """Fragments to scatter mm1's natural PSUM M-tile m of block k (padded
    rows [128m, 128m+128)) into the plane layout.

    Returns [(src_part0, size, plane_l, plane_tile, dst_part_base), ...].
    Fragment boundaries lie on the src 128-grid, dst 128-grid and l-chunk
    grid; shifted fragments are split to the DVE-legal 64 (or 32) grain.
    """
    grain = 64 if chunk % 64 == 0 else 32
    frags = []
    g = 128 * m
    end = 128 * (m + 1)
    while g < end:
        l = g // chunk
        dst = k * chunk + (g - l * chunk)  # global row within plane l
        nb_ = min(end, (l + 1) * chunk, g + (128 - dst % 128))
        size = nb_ - g
        src_b = g - 128 * m
        dst_b = dst % 128
        if src_b % 128 == dst_b:
            frags.append((src_b, size, l, dst // 128, dst_b))
            g = nb_
        else:
            step = min(grain, size)
            frags.append((src_b, step, l, dst // 128, dst_b))
            g += step
    return frags


def ktiles(p_dim):
    """[(row0, size), ...] 128-partition contraction tiles covering p_dim."""
    return [(r, min(128, p_dim - r)) for r in range(0, p_dim, 128)]


def prepare_weights(inputs):
    """Host-side arrangement of all weights/biases into DRAM-parameter arrays.

    Weights are pre-swizzled into FLAT images matching the exact SBUF layout
    of each const tile, so the device loads them with a handful of large
    (~1 MB) DMAs instead of ~96 small ones: small HBM transfers are
    descriptor-dominated (64 KB -> ~1/3 of peak), and the serial weight
    stream on one HWDGE ring was the bulk of the invocation's startup cost.
    """
    npdt = _np_mmdt()
    arrs = {}
    for li, (p_dim, q_dim, chunk, s_dim) in enumerate(LAYER_CFG, 1):
        w1 = np.asarray(inputs[f"w1_{li}"], np.float32)
        w2 = np.asarray(inputs[f"w2_{li}"], np.float32)
        w1t, w2t = arrange_layer(w1, w2, q_dim, chunk)
        QPAD = 4 * chunk
        kts = ktiles(p_dim)
        nkt = len(kts)
        nrt = QPAD // 128
        # w1 flat image: [4, 128, nkt*QPAD]; block k chunk is contiguous
        w1f = np.zeros((4, 128, nkt * QPAD), np.float32)
        for k in range(4):
            for ki, (k0, ksz) in enumerate(kts):
                w1f[k, :ksz, ki * QPAD : (ki + 1) * QPAD] = w1t[k, k0 : k0 + ksz, :]
        arrs[f"w1f_{li}"] = w1f.astype(npdt)
        # w2 flat image: [128, 4*nrt*s_dim] (exact SBUF layout, one DMA)
        w2f = np.zeros((128, 4 * nrt * s_dim), np.float32)
        for l in range(4):
            for rt in range(nrt):
                col = (l * nrt + rt) * s_dim
                w2f[:, col : col + s_dim] = w2t[l, 128 * rt : 128 * (rt + 1), :]
        arrs[f"w2f_{li}"] = w2f.astype(npdt)
        bias = np.asarray(inputs[f"b{li}"], np.float32)  # [4*s_dim], f'=l*s+s
        mts = ktiles(s_dim)
        cols = np.zeros((128, 4 * len(mts)), np.float32)
        for l in range(4):
            for mi, (m0, msz) in enumerate(mts):
                cols[:msz, l * len(mts) + mi] = bias[l * s_dim + m0 : l * s_dim + m0 + msz]
        arrs[f"bias_{li}"] = cols

    # L3: dense composite [100, 1000]; lhsT tiles in h3's (l, mi) feature order
    M3 = compose_monarch(np.asarray(inputs["w1_3"], np.float32),
                         np.asarray(inputs["w2_3"], np.float32), 100)
    mts3 = ktiles(250)
    w3 = np.zeros((8, 128, 100), np.float32)
    for l in range(4):
        for mi, (m0, msz) in enumerate(mts3):
            w3[l * 2 + mi, :msz, :] = M3[:, 250 * l + m0 : 250 * l + m0 + msz].T
    # L4: dense composite [10, 100], stored transposed as mm rhs
    M4 = compose_monarch(np.asarray(inputs["w1_4"], np.float32),
                         np.asarray(inputs["w2_4"], np.float32), 10)
    # packed small-constant image (MM_DT): w3 tiles | m4t | b4 row
    cstw = np.zeros((128, 8 * 100 + 10 + 10), np.float32)
    for ti in range(8):
        cstw[:, ti * 100 : (ti + 1) * 100] = w3[ti]
    cstw[:100, 800:810] = M4.T
    cstw[0, 810:820] = np.asarray(inputs["b4"], np.float32)
    arrs["cstw"] = cstw.astype(npdt)
    # packed f32 bias image: bias_1 (24) | bias_2 (8) | b3 col (1)
    cstb = np.zeros((128, 33), np.float32)
    cstb[:, 0:24] = arrs.pop("bias_1")
    cstb[:, 24:32] = arrs.pop("bias_2")
    cstb[:100, 32] = np.asarray(inputs["b3"], np.float32)
    arrs["cstb"] = cstb
    return arrs


def make_in_maps(inputs):
    """Per-core input maps: shared weight arrays + per-core pre-transposed x."""
    npdt = _np_mmdt()
    arrs = prepare_weights(inputs)
    x = np.asarray(inputs["x"], np.float32)
    n_bt = B_CORE // NB
    in_maps = []
    for c in range(N_CORES):
        xc = x[c * B_CORE : (c + 1) * B_CORE].astype(npdt)  # [2048, 3072]
        xt = np.ascontiguousarray(
            xc.T.reshape(3072, n_bt, NB).transpose(1, 0, 2))  # [n_bt, 3072, NB]
        m = dict(arrs)
        m["xt"] = xt
        in_maps.append(m)
    return in_maps


def build_nc(b_core=B_CORE, repeat=1, reload_weights=False):
    """Build the single-core Bass program (SPMD: same program, per-core xt).
    repeat>1 re-runs the whole batch pipeline (for timing-by-differencing);
    reload_weights=True also re-DMAs all weights each pass so the marginal
    pass models a full cold invocation (what the grader times)."""
    nc = bacc.Bacc(None, target_bir_lowering=False)
    n_bt = b_core // NB
    xt_d = nc.declare_dram_parameter("xt", [n_bt, 3072, NB], MM_DT, isOutput=False)
    y_d = nc.declare_dram_parameter("y", [b_core, 10], F32, isOutput=True)

    wd = {}
    for li, (p_dim, q_dim, chunk, s_dim) in enumerate(LAYER_CFG, 1):
        QPAD = 4 * chunk
        nkt = len(ktiles(p_dim))
        nrt = QPAD // 128
        wd[f"w1f_{li}"] = nc.declare_dram_parameter(
            f"w1f_{li}", [4, 128, nkt * QPAD], MM_DT, isOutput=False)
        wd[f"w2f_{li}"] = nc.declare_dram_parameter(
            f"w2f_{li}", [128, 4 * nrt * s_dim], MM_DT, isOutput=False)
    wd["cstw"] = nc.declare_dram_parameter("cstw", [128, 820], MM_DT, isOutput=False)
    wd["cstb"] = nc.declare_dram_parameter("cstb", [128, 33], F32, isOutput=False)

    with tile.TileContext(nc) as tc:
        with (
            tc.tile_pool(name="const", bufs=1) as const,
            tc.tile_pool(name="xts", bufs=2) as xts,
            tc.tile_pool(name="acts", bufs=1) as acts,
            tc.tile_pool(name="psum_mm", bufs=7, space="PSUM") as psum_mm,
            tc.tile_pool(name="psum_s", bufs=1, space="PSUM") as psum_s,
            tc.tile_pool(name="sm", bufs=2) as smp,
        ):
            # ---- resident constants ----
            ones_row = const.tile([1, 128], MM_DT, name="ones_row", tag="ones_row")
            nc.any.memset(ones_row[:], 1.0)

            w1sb, w2sb, biassb = {}, {}, {}
            for li, (p_dim, q_dim, chunk, s_dim) in enumerate(LAYER_CFG, 1):
                QPAD = 4 * chunk
                nkt = len(ktiles(p_dim))
                nrt = QPAD // 128
                w1sb[li] = const.tile([128, nkt * 4 * QPAD], MM_DT,
                                      name=f"w1sb{li}", tag=f"w1sb{li}")
                w2sb[li] = const.tile([128, 4 * nrt * s_dim], MM_DT,
                                      name=f"w2sb{li}", tag=f"w2sb{li}")
            cstw = const.tile([128, 820], MM_DT, name="cstw", tag="cstw")
            cstb = const.tile([128, 33], F32, name="cstb", tag="cstb")
            w3sb = cstw[:, 0:800]
            m4sb = cstw[:, 800:810]
            b4sb = cstw[0:1, 810:820]
            biassb[1] = cstb[:, 0:24]
            biassb[2] = cstb[:, 24:32]
            b3sb = cstb[:, 32:33]

            def emit_weight_loads():
                """Big-chunk weight DMAs split across two DMA rings so the
                stream hides under tile-0 compute.  ACT HWDGE ring: L1 w1
                per block k (~1.2 MB each, consumed in that order by mm1).
                gpsimd SWDGE ring: biases first (needed by the first relu
                eviction), then L1 w2 as one 4.6 MB DMA (needed when mm2
                starts ~15 us in), then L2 weights and the packed consts."""
                c1 = 6 * 768
                for k in range(4):
                    nc.scalar.dma_start(
                        w1sb[1][:, k * c1 : (k + 1) * c1],
                        wd["w1f_1"][k, :, :])
                nc.gpsimd.dma_start(cstb[:], wd["cstb"][:, :])
                nc.gpsimd.dma_start(w2sb[1][:], wd["w2f_1"][:, :])
                c1 = 6 * 256
                for k in range(0, 4, 2):  # two 0.8 MB chunks, off the ACT queue
                    nc.gpsimd.dma_start(
                        w1sb[2][:, k * c1 : (k + 2) * c1].rearrange(
                            "p (k c) -> p k c", k=2),
                        wd["w1f_2"][k : k + 2, :, :].rearrange("k p c -> p k c"))
                nc.gpsimd.dma_start(w2sb[2][:], wd["w2f_2"][:, :])
                nc.gpsimd.dma_start(cstw[:], wd["cstw"][:, :])

            emit_weight_loads()

            # Pre-warm the PE clock gate (HAM) with dummy matmuls on scratch
            # data during the otherwise-idle first-weight-chunk window, so
            # the real tile-0 matmuls start at full clock.
            warm = const.tile([128, 512], MM_DT, name="warm", tag="warm")
            nc.vector.memset(warm[:], 0.0)
            ps_w = psum_mm.tile([128, NB], F32, name="ps_mm", tag="ps_mm")
            for i in range(12):
                nc.tensor.matmul(
                    ps_w[:, :], warm[:, :128], warm[:, :],
                    start=(i == 0), stop=(i == 11))

            nsub = NB // 128

            def emit_relu(dst, ps, bias_ap, eng):
                """relu(x + bias) eviction on the chosen engine."""
                if eng == "A":
                    nc.scalar.activation(
                        dst, ps, mybir.ActivationFunctionType.Relu, bias=bias_ap)
                else:
                    e = nc.vector if eng == "V" else nc.gpsimd
                    e.tensor_scalar(
                        dst, ps, bias_ap, 0.0,
                        mybir.AluOpType.add, mybir.AluOpType.max)

            def emit_tail(bt, h4):
                """L4 (operand-swapped, batch-major) + log_softmax + store."""
                ps4 = psum_s.tile([128, nsub * 10], F32, name="ps4", tag="ps4")
                for s in range(nsub):
                    nc.tensor.matmul(
                        ps4[:, 10 * s : 10 * s + 10],
                        h4[:100, 128 * s : 128 * s + 128],
                        m4sb[:100, :],
                        start=True, stop=False,
                    )
                    nc.tensor.matmul(
                        ps4[:, 10 * s : 10 * s + 10],
                        ones_row[:1, :],
                        b4sb[:1, :],
                        start=False, stop=True,
                    )
                # log_softmax without the max-shift: logits are bounded
                # (|h4| <~ 7), so fp32 exp/sum cannot overflow and the
                # shorter serial chain trims the exposed final-tile tail.
                ex = smp.tile([128, nsub * 10], F32, name="ex", tag="ex")
                for s in range(nsub):
                    nc.scalar.activation(
                        ex[:, 10 * s : 10 * s + 10], ps4[:, 10 * s : 10 * s + 10],
                        mybir.ActivationFunctionType.Exp)
                ex3 = ex.rearrange("p (s c) -> p s c", c=10)
                sme = smp.tile([128, nsub], F32, name="sme", tag="sme")
                nc.vector.reduce_sum(sme[:], ex3, axis=mybir.AxisListType.X)
                lse = smp.tile([128, nsub], F32, name="lse", tag="lse")
                nc.scalar.activation(
                    lse[:], sme[:], mybir.ActivationFunctionType.Ln)
                ofs = smp.tile([128, nsub], F32, name="ofs", tag="ofs")
                nc.scalar.mul(ofs[:], lse[:], -1.0)
                out_t = smp.tile([128, nsub * 10], F32, name="out_t", tag="out_t")
                for s in range(nsub):
                    nc.scalar.activation(
                        out_t[:, 10 * s : 10 * s + 10], ps4[:, 10 * s : 10 * s + 10],
                        mybir.ActivationFunctionType.Identity,
                        bias=ofs[:, s : s + 1])
                    row0 = bt * NB + s * 128
                    nc.sync.dma_start(
                        y_d[row0 : row0 + 128, :], out_t[:, 10 * s : 10 * s + 10])

            # ---- batch-tile pipeline ----
            inlane_rr = [0]  # round-robin among Pool / scheduler-chosen
            pending = None  # (bt, h4) whose L4+softmax tail is deferred
            for rep_bt in [(r, t) for r in range(repeat) for t in range(n_bt)]:
                rep, bt = rep_bt
                if reload_weights and rep > 0 and bt == 0:
                    emit_weight_loads()
                # x tile: feature-major bf16, one DMA per block k so L1 mm1
                # of block k only waits on its own quarter.
                xT = xts.tile([128, 24 * NB], MM_DT, name="xT", tag="xT")
                xTr = xT.rearrange("p (g b) -> p g b", b=NB)
                for k in range(4):
                    nc.sync.dma_start(
                        xTr[:, 6 * k : 6 * k + 6, :],
                        xt_d[bt, 768 * k : 768 * (k + 1), :].rearrange(
                            "(g p) b -> p g b", p=128),
                    )

                # --- L1 mm1: natural block M-tiles; evictions scatter
                # fragments into the plane layout via DVE copies.
                p_dim, q_dim, chunk, s_dim = LAYER_CFG[0]
                QPAD = 4 * chunk
                ntl = QPAD // 128
                kts = ktiles(p_dim)
                nkt = len(kts)
                planes = acts.tile([128, 4 * ntl * NB], MM_DT,
                                   name="planes1", tag="planes1")
                # k-major: tile-0 consumes each ~1.2 MB w1 chunk for ~4 us
                # while the next one streams, instead of stalling on all four
                # within the first 2.6 us of PE work (cold-start pacing).
                for k in range(4):
                    for m in range(ntl):
                        ps = psum_mm.tile([128, NB], F32, name="ps_mm", tag="ps_mm")
                        for ki, (k0, ksz) in enumerate(kts):
                            wcol = (k * nkt + ki) * QPAD + 128 * m
                            nc.tensor.matmul(
                                ps[:, :],
                                w1sb[1][:ksz, wcol : wcol + 128],
                                xTr[:, 6 * k + ki, :],
                                start=(ki == 0),
                                stop=(ki == nkt - 1),
                            )
                        for (s0, sz, l, jt, db) in evict_frags(k, m, chunk):
                            pcol = (l * ntl + jt) * NB
                            eng = nc.vector if s0 % 128 != db else nc.any
                            eng.tensor_copy(
                                planes[db : db + sz, pcol : pcol + NB],
                                ps[s0 : s0 + sz, :],
                            )
                        # previous tile's serial tail rides early in this
                        # tile's mm1 phase, where ACT/DVE queues are idle
                        if pending is not None:
                            emit_tail(*pending)
                            pending = None

                # --- L1 mm2: planes -> h2 blocks (relu+bias on evict)
                mts = ktiles(s_dim)
                nmt = len(mts)
                h = acts.tile([128, 4 * nmt * NB], MM_DT, name="h2", tag="h2")
                for l in range(4):
                    for mi, (m0, msz) in enumerate(mts):
                        ps = psum_mm.tile([128, NB], F32, name="ps_mm", tag="ps_mm")
                        for rt in range(ntl):
                            wcol = (l * ntl + rt) * s_dim + m0
                            nc.tensor.matmul(
                                ps[:msz, :],
                                w2sb[1][:, wcol : wcol + msz],
                                planes[:, (l * ntl + rt) * NB : (l * ntl + rt + 1) * NB],
                                start=(rt == 0),
                                stop=(rt == ntl - 1),
                            )
                        hcol = (l * nmt + mi) * NB
                        emit_relu(h[:msz, hcol : hcol + NB], ps[:msz, :],
                                  biassb[1][:msz, l * nmt + mi : l * nmt + mi + 1],
                                  L1_RELU_ENG[l * nmt + mi])
                in_tiles = [
                    [((l * nmt + mi) * NB, msz) for mi, (m0, msz) in enumerate(mts)]
                    for l in range(4)
                ]

                # --- L2 mm1 + mm2 (same structure, smaller)
                p_dim, q_dim, chunk, s_dim = LAYER_CFG[1]
                QPAD = 4 * chunk
                ntl = QPAD // 128
                kts = ktiles(p_dim)
                nkt = len(kts)
                planes2 = acts.tile([128, 4 * ntl * NB], MM_DT,
                                    name="planes2", tag="planes2")
                for m in range(ntl):
                    for k in range(4):
                        ps = psum_mm.tile([128, NB], F32, name="ps_mm", tag="ps_mm")
                        for ki, (k0, ksz) in enumerate(kts):
                            hcol = in_tiles[k][ki][0]
                            wcol = (k * nkt + ki) * QPAD + 128 * m
                            nc.tensor.matmul(
                                ps[:, :],
                                w1sb[2][:ksz, wcol : wcol + 128],
                                h[:ksz, hcol : hcol + NB],
                                start=(ki == 0),
                                stop=(ki == nkt - 1),
                            )
                        for (s0, sz, l, jt, db) in evict_frags(k, m, chunk):
                            pcol = (l * ntl + jt) * NB
                            eng = nc.vector if s0 % 128 != db else nc.any
                            eng.tensor_copy(
                                planes2[db : db + sz, pcol : pcol + NB],
                                ps[s0 : s0 + sz, :],
                            )
                mts = ktiles(s_dim)
                nmt = len(mts)
                h3 = acts.tile([128, 4 * nmt * NB], MM_DT, name="h3", tag="h3")
                for l in range(4):
                    for mi, (m0, msz) in enumerate(mts):
                        ps = psum_mm.tile([128, NB], F32, name="ps_mm", tag="ps_mm")
                        for rt in range(ntl):
                            wcol = (l * ntl + rt) * s_dim + m0
                            nc.tensor.matmul(
                                ps[:msz, :],
                                w2sb[2][:, wcol : wcol + msz],
                                planes2[:, (l * ntl + rt) * NB : (l * ntl + rt + 1) * NB],
                                start=(rt == 0),
                                stop=(rt == ntl - 1),
                            )
                        hcol = (l * nmt + mi) * NB
                        emit_relu(h3[:msz, hcol : hcol + NB], ps[:msz, :],
                                  biassb[2][:msz, l * nmt + mi : l * nmt + mi + 1],
                                  L2_RELU_ENG[l * nmt + mi])
                h3_tiles = [((l * nmt + mi) * NB, msz)
                            for l in range(4) for mi, (m0, msz) in enumerate(mts)]

                # --- L3: dense composite, one [100, NB] accumulation
                ps3 = psum_mm.tile([128, NB], F32, name="ps_mm", tag="ps_mm")
                for ti, (hcol, ksz) in enumerate(h3_tiles):
                    nc.tensor.matmul(
                        ps3[:100, :],
                        w3sb[:ksz, ti * 100 : ti * 100 + 100],
                        h3[:ksz, hcol : hcol + NB],
                        start=(ti == 0),
                        stop=(ti == 7),
                    )
                h4 = acts.tile([128, NB], MM_DT, name="h4", tag="h4")
                nc.scalar.activation(
                    h4[:100, :], ps3[:100, :],
                    mybir.ActivationFunctionType.Relu,
                    bias=b3sb[:100, 0:1])
                pending = (bt, h4)
            emit_tail(*pending)
    nc.compile()
    return nc


def kernel(**inputs):
    in_maps = make_in_maps(inputs)
    nc = build_nc()
    res = run_bass_kernel_spmd(nc, in_maps, list(range(N_CORES))).results
    return np.concatenate([r["y"] for r in res], axis=0)

